# revision 2
# baseline (speedup 1.0000x reference)
"""CASSI forward A^T(A(x)) kernel for Trainium2, 8-core data parallel.

Reference computation (independent per batch b and row m):
    y1[l, n]  = x[b, l, m, n] * phi[l, m, n]
    y2[j]     = sum_l y1[l, j - 2l]              (j in [0, 310))
    out[l, n] = phi[l, m, n] * y2[2l + n]

On-chip layout: partitions = rows m (two 128-row tiles per batch), free
dim = (l, n).  The 28-band shift-scatter-add runs as a 5-level binary tree
of strided DVE adds over a scratch tile laid out with small zero gaps
between paired bands, so each tree level is a single wide strided
tensor_tensor op whose shifted operand reads zeros where a block has no
data.  Gaps are memset once at kernel start; level ops rewrite only data
regions.

Uniform-slot layout: at every level, slot width = data width + next-level
shift, so in0's right-pad zeros and in1's left-pad zeros are the SAME gap
cells and every level op is a plain 2-free-dim strided tensor_tensor:
  y1  band l (256) at 258*l                        gaps [256,258) per slot
  u   i=0..13 (258) at 262*i                       gaps [258,262)
  q   i=0..6  (262) at 278*i                       gaps [262,278), [1930,1938)
  o   i=0..2  (270) at 286*i                       gaps [270,286), [842,850)
  s   s0 (286) at 0, m1 (278) at 342               zeros [286,342)
  y2  (310) dense

Precision/layout strategy: all HBM traffic is fp16 (inputs are cast and
row-major-transposed to [.., M, L, N] on the host inside kernel(), the
output is cast/transposed back).  This halves HBM bytes (the kernel is
memory-bound) and makes every DMA a dense fully-contiguous ~1.8 MB
transfer, and fp16 tensor_tensor runs in the DVE 2x perf mode.  All
element offsets in the scratch layouts are even, keeping every DVE
operand 4-byte aligned as the 2x mode requires.  Accumulating 28 bands
in fp16 keeps worst-case relative error ~1e-3, far inside the 2e-2 gate.

Sharding: batch dim (32) split 4-per-core across 8 cores; phi replicated.
"""

import numpy as np

B, L, M, N = 32, 28, 256, 256
STRIDE = 2
NCORES = 8
BPC = B // NCORES            # batches per core
NOUT = N + STRIDE * (L - 1)  # 310
P = 128                      # partitions per row tile
LN = L * N                   # 7168
Y1_W = 258 * 28              # 7224, band l at 258*l, gaps [256,258) per slot
U_W = 262 * 14               # 3668, u_i at 262*i, gaps [258,262)
Q_W = 1938                   # q_i at 278*i (uniform); gaps [262,278), [1930,1938)
O_W = 850                    # o_i at 286*i; zeros [270,286)x2, [842,850)
S_W = 620                    # s0@0 (286), zeros [286,342), m1@342 (278)

_cached = {}


def _build_nc():
    import concourse.bass as bass
    import concourse.mybir as mybir
    from concourse.ap import AP
    from concourse.tile import TileContext

    f16 = mybir.dt.float16
    nc = bass.Bass()
    x = nc.dram_tensor("x", [BPC, M, LN], f16, kind="ExternalInput")
    phi = nc.dram_tensor("phi", [M, LN], f16, kind="ExternalInput")
    out = nc.dram_tensor("out", [BPC, M, LN], f16, kind="ExternalOutput")

    def sub(t, off, dims):
        """AP over tile t at element offset off with free dims [[step,count],..]."""
        full = t[:]
        return AP(full.tensor, full.offset + off,
                  [[full.ap[0][0], P]] + [list(d) for d in dims])

    with TileContext(nc) as tc:
        with (
            tc.tile_pool(name="phipool", bufs=1) as phipool,
            tc.tile_pool(name="xpool", bufs=1) as xpool,
            tc.tile_pool(name="scratch", bufs=1) as sp,
        ):
            # --- persistent tiles ------------------------------------------------
            phit = [phipool.tile([P, LN], f16, name=f"phi{pt}", tag=f"phi{pt}")
                    for pt in range(M // P)]
            xts = [xpool.tile([P, LN], f16, name=f"xt{i}", tag=f"xt{i}")
                   for i in range(2)]
            ots_ = [xpool.tile([P, LN], f16, name=f"ou{i}", tag=f"ou{i}")
                    for i in range(2)]
            y1t = sp.tile([P, Y1_W], f16, name="y1", tag="y1")
            ut = sp.tile([P, U_W], f16, name="u", tag="u")
            qt = sp.tile([P, Q_W], f16, name="q", tag="q")
            ot = sp.tile([P, O_W], f16, name="o", tag="o")
            st = sp.tile([P, S_W], f16, name="s", tag="s")
            y2t = sp.tile([P, NOUT], f16, name="y2", tag="y2")

            # --- one-time zero-gap memsets (never written afterwards) ------------
            nc.vector.memset(sub(y1t, 256, [[258, 28], [1, 2]]), 0.0)
            nc.vector.memset(sub(ut, 258, [[262, 14], [1, 4]]), 0.0)
            nc.vector.memset(sub(qt, 262, [[278, 6], [1, 16]]), 0.0)
            nc.vector.memset(sub(qt, 1930, [[1, 8]]), 0.0)
            nc.vector.memset(sub(ot, 270, [[286, 2], [1, 16]]), 0.0)
            nc.vector.memset(sub(ot, 842, [[1, 8]]), 0.0)
            nc.vector.memset(sub(st, 286, [[1, 56]]), 0.0)

            # --- phi tile 0 load (SP ring; ACT ring starts on x loads) -----------
            nc.sync.dma_start(out=phit[0][:], in_=phi[0:P])

            it = 0
            for pt in range(M // P):
                for b in range(BPC):
                    xt = xts[it % 2]
                    outt = ots_[it % 2]
                    it += 1
                    if it == 2:
                        # phi1 load deferred past startup so it doesn't steal
                        # HBM bandwidth from phi0/load0
                        nc.scalar.dma_start(out=phit[1][:], in_=phi[P: 2 * P])
                    nc.scalar.dma_start(
                        out=xt[:], in_=x[b][pt * P: (pt + 1) * P],
                    )
                    # y1 = x * phi, dense -> uniform gapped scratch
                    nc.vector.tensor_mul(
                        out=sub(y1t, 0, [[258, 28], [1, 256]]),
                        in0=sub(xt, 0, [[256, 28], [1, 256]]),
                        in1=sub(phit[pt], 0, [[256, 28], [1, 256]]),
                    )
                    # L1: 14 pair-sums -> u
                    nc.vector.tensor_add(
                        out=sub(ut, 0, [[262, 14], [1, 258]]),
                        in0=sub(y1t, 0, [[516, 14], [1, 258]]),
                        in1=sub(y1t, 256, [[516, 14], [1, 258]]),
                    )
                    # L2: 7 quad-sums -> q (single uniform op, stride 278)
                    nc.vector.tensor_add(
                        out=sub(qt, 0, [[278, 7], [1, 262]]),
                        in0=sub(ut, 0, [[524, 7], [1, 262]]),
                        in1=sub(ut, 258, [[524, 7], [1, 262]]),
                    )
                    # L3: 3 oct-sums -> o
                    nc.vector.tensor_add(
                        out=sub(ot, 0, [[286, 3], [1, 270]]),
                        in0=sub(qt, 0, [[556, 3], [1, 270]]),
                        in1=sub(qt, 270, [[556, 3], [1, 270]]),
                    )
                    # L4: s0 = o0 + shift16(o1); m1 = o2 + shift16(q6)
                    nc.vector.tensor_add(
                        out=sub(st, 0, [[1, 286]]),
                        in0=sub(ot, 0, [[1, 286]]),
                        in1=sub(ot, 270, [[1, 286]]),
                    )
                    nc.vector.tensor_add(
                        out=sub(st, 342, [[1, 278]]),
                        in0=sub(ot, 572, [[1, 278]]),
                        in1=sub(qt, 1652, [[1, 278]]),
                    )
                    # L5: y2 = s0 + shift32(m1)
                    nc.vector.tensor_add(
                        out=sub(y2t, 0, [[1, 310]]),
                        in0=sub(st, 0, [[1, 310]]),
                        in1=sub(st, 310, [[1, 310]]),
                    )
                    # out = phi * gather(y2) into a dense tile so the store
                    # below is a single fully-contiguous transfer
                    nc.vector.tensor_mul(
                        out=sub(outt, 0, [[256, 28], [1, 256]]),
                        in0=sub(y2t, 0, [[2, 28], [1, 256]]),
                        in1=sub(phit[pt], 0, [[256, 28], [1, 256]]),
                    )
                    o_hbm = out[b][pt * P: (pt + 1) * P]
                    if it < 2 * BPC:
                        # full store on the SP ring (ACT ring carries loads)
                        nc.sync.dma_start(out=o_hbm, in_=outt[:])
                    else:
                        # last store split across both rings to halve the
                        # tail drain
                        half = LN // 2
                        for par, eng in ((0, nc.sync), (1, nc.scalar)):
                            eng.dma_start(
                                out=AP(o_hbm.tensor, o_hbm.offset + half * par,
                                       [list(o_hbm.ap[0]), [1, half]]),
                                in_=sub(outt, half * par, [[1, half]]),
                            )
    _split_excess_waits(nc, mybir)
    return nc


def _split_excess_waits(nc, mybir):
    """Move all-but-one semaphore waits off capacity-limited instructions.

    The TRN2 ISA packs sync commands into each 64B instruction; multi-dim
    TT/DMA encodings have room for only one wait, and walrus codegen dies
    with "Too many sync wait commands" instead of splitting.  A standalone
    EventSemaphore on the same engine right before the op is semantically
    identical (the sequencer executes both in order)."""
    ctr = 0
    for bb in nc.m.functions[0].blocks:
        new = []
        for ins in bb.instructions:
            si = ins.sync_info
            waits = list(si.on_wait) if si is not None and si.on_wait else []
            if len(waits) > 1:
                for w in waits[:-1]:
                    ctr += 1
                    new.append(mybir.InstEventSemaphore(
                        name=f"wsplit-{ctr}",
                        engine=ins.engine,
                        sync_info=mybir.SyncInfo(on_wait=[w], on_update=[]),
                    ))
                ins.sync_info = mybir.SyncInfo(
                    on_wait=[waits[-1]],
                    on_update=list(si.on_update or []),
                )
            new.append(ins)
        bb.instructions = new


def _get_nc():
    if "nc" not in _cached:
        _cached["nc"] = _build_nc()
    return _cached["nc"]


def _prep_in_maps(x: np.ndarray, phi: np.ndarray) -> list[dict]:
    """Shard batch across cores; cast to fp16 and transpose to [.., M, L, N]
    row-major so every device DMA is dense and contiguous."""
    phi_t = phi.transpose(1, 0, 2).reshape(M, LN).astype(np.float16, order="C")
    in_maps = []
    for c in range(NCORES):
        xs = (x[c * BPC: (c + 1) * BPC]
              .transpose(0, 2, 1, 3)
              .reshape(BPC, M, LN)
              .astype(np.float16, order="C"))
        in_maps.append({"x": xs, "phi": phi_t})
    return in_maps


def _postprocess(outs: list[np.ndarray]) -> np.ndarray:
    """Invert the device layout: fp16 [BPC, M, L*N] shards -> f32 [B,L,M,N]."""
    full = np.empty((B, L, M, N), dtype=np.float32)
    for c, o in enumerate(outs):
        o = np.asarray(o).reshape(BPC, M, L, N).astype(np.float32)
        full[c * BPC: (c + 1) * BPC] = o.transpose(0, 2, 1, 3)
    return full


def kernel(x: np.ndarray, phi: np.ndarray) -> np.ndarray:
    from concourse.bass_utils import run_bass_kernel_spmd

    x = np.asarray(x, dtype=np.float32)
    phi = np.asarray(phi, dtype=np.float32)
    assert x.shape == (B, L, M, N) and phi.shape == (L, M, N)

    nc = _get_nc()
    in_maps = _prep_in_maps(x, phi)
    res = run_bass_kernel_spmd(nc, in_maps, core_ids=list(range(NCORES)))
    return _postprocess([res.results[c]["out"] for c in range(NCORES)])
